# revision 49
# baseline (speedup 1.0000x reference)
"""Cross_Atten_Lite_split Trainium2 Bass kernel (v2).

Sharding: 8 cores = (batch b in 0..3) x (query-half qh in 0..1).
Each core computes both attention heads for 2048 queries x 4096 keys of
its batch. No collectives.

Math rewrites (validated vs reference):
  - eval-mode BN on x1/x2 folded into kq1_w/kq2_w (+bias).
  - channel_shuffle is a permutation of the shared q/k contraction axis
    -> eliminated;  k_h = [kq1[:,64h:64h+32]; kq2[:,64h:64h+32]],
    q_h likewise from rows 64h+32:64h+64.
  - K bias cancels in softmax (per-query constant); dropped.
  - V bias commutes out of attention (softmax rows sum to 1):
    folded into out bias on host (bout += Wout @ v_b).
  - final BN + w_scale folded into out_w/out_b.
  - softmax without max-subtraction (fp32 range safe for this data).
  - softmax denominator via shared ones-column in Vtok ([v1|1|v2]).

Performance structure (227us baseline -> ~145us):
  - inputs x/x1/x2 + all weights in bf16; weights packed into ONE dram
    tensor / one DMA; inputs in graduated chunks (1/1/2/4 tiles) over
    both HWDGE queues (DGE generation is ~625ns/DMA and serializes).
  - V computed pre-transposed: x_tile^T @ w matmuls (bf16 full rate),
    no transpose pass; V bias folded into out bias on host.
  - projections (phase A) merged with attention tile j=0: as input
    tile t lands, S/exp/PV slices m=4t..4t+3 run for both heads, with
    tile t+1's projections+KT copies issued mid-stream.
  - exp split across Act (exact table exp -> bf16) and DVE (Schraudolph
    int16 trick: i16 = s*EA + EB, bitcast bf16), ~18/14 per segment;
    segment-lead and post-C slices pinned to Act so DVE normalize/STT
    bursts never stall PV.
  - per-slice software pipeline, S(m+4) issued ahead of PV(m);
    PSUM: 5 sp + 2 op + 1 pp banks.
  - lazy h1 normalize (issued inside the next segment); output
    projection of tile j issued inside tile j+1's m-loop.
  - softmax reciprocal broadcast via gpsimd partition_broadcast.
"""

import numpy as np
from contextlib import ExitStack

import concourse.bass as bass
import concourse.bacc as bacc
import concourse.mybir as mybir
import concourse.tile as tile
from concourse.bass_utils import run_bass_kernel_spmd

F32 = mybir.dt.float32
F32R = mybir.dt.float32r
I32 = mybir.dt.int32
I16 = mybir.dt.int16
BF16 = mybir.dt.bfloat16
AF = mybir.ActivationFunctionType
ALU = bass.mybir.AluOpType

C = 256          # channels (INC1 == INC2)
N = 4096         # tokens per batch (64*64)
NQ = 2048        # queries per core
NT = 512         # free-dim tile size

# Schraudolph exp-approx constants (bf16 output):
# exp(0.125*s) ~= bf16_bits(i16(s*EA + EB))
EA = 0.125 * 128.0 * 1.4426950408889634  # 0.125 * 2^7 * log2(e)
EB = 127.0 * 128.0 - 366392.0 / 65536.0  # max-rel balanced Schraudolph C

# exp slice engine split per (j,h): True -> Act, False -> DVE.
# First 4 slices go to Act (DVE handles the lazy normalize of the previous
# segment there); slices 7/8 and 15/16 go to Act in segments that also issue
# the previous tile's output projection (STT bursts land on DVE there).
def _mk_eng(act_n, n, head_act, extra_act=()):
    eng = [False] * n
    for k in head_act:
        eng[k] = True
    for k in extra_act:
        eng[k] = True
    need = act_n - sum(eng)
    free = [k for k in range(n) if not eng[k]]
    acc = 0.0
    for k in free:
        acc += need / float(len(free))
        if acc >= 1.0:
            acc -= 1.0
            eng[k] = True
    return eng

_ENG = _mk_eng(19, 32, (0, 1, 2, 3, 4))
_ENG_C = _mk_eng(21, 32, (0, 1, 2, 3, 4), (7, 8, 15, 16))


def build_bass():
    nc = bacc.Bacc("TRN2", target_bir_lowering=False, debug=False, num_devices=8)

    x1T = nc.dram_tensor("x1T", [C, N], BF16, kind="ExternalInput").ap()
    x2T = nc.dram_tensor("x2T", [C, N], BF16, kind="ExternalInput").ap()
    xT = nc.dram_tensor("xT", [C, N], BF16, kind="ExternalInput").ap()
    wall = nc.dram_tensor("wall", [8, 128, 128], BF16, kind="ExternalInput").ap()
    ball = nc.dram_tensor("ball", [3, 128], F32, kind="ExternalInput").ap()
    outT = nc.dram_tensor("outT", [C, NQ], BF16, kind="ExternalOutput").ap()

    with ExitStack() as ctx:
        tc = ctx.enter_context(tile.TileContext(nc))
        const = ctx.enter_context(tc.tile_pool(name="const", bufs=1))
        pers = ctx.enter_context(tc.tile_pool(name="pers", bufs=1))

        wcat = const.tile([128, 1024], BF16, name="wcat")
        bcat = const.tile([128, 3], F32, name="bcat")

        # kq weights first (gate the very first matmul), rest after the
        # first input chunks
        nc.sync.dma_start(out=wcat[:, 0:512],
                          in_=wall[0:4].rearrange("s p c -> p s c"))
        w_kq1 = [wcat[:, 128 * g:128 * (g + 1)] for g in range(2)]
        w_kq2 = [wcat[:, 256 + 128 * g:256 + 128 * (g + 1)] for g in range(2)]
        w_vT = [wcat[:, 512 + 128 * g:512 + 128 * (g + 1)] for g in range(2)]
        w_out = [wcat[:, 768 + 128 * g:768 + 128 * (g + 1)] for g in range(2)]
        b_q = bcat[:, 0:1]
        b_out = [bcat[:, 1 + g:2 + g] for g in range(2)]

        # persistent SBUF (g-halves side by side: cols g*N + 0..N)
        x1sb = pers.tile([128, 2 * N], BF16, name="x1sb")
        x2sb = pers.tile([128, 2 * N], BF16, name="x2sb")
        xsb = pers.tile([128, 2 * N], BF16, name="xsb")
        KT = pers.tile([128, N], F32R, name="KT")
        QT = pers.tile([128, NQ], F32R, name="QT")
        Vtok = pers.tile([128, 32 * 130], BF16, name="Vtok")
        Ocat = pers.tile([128, NQ], BF16, name="Ocat")

        vt4 = Vtok.rearrange("p (m two c) -> p m two c", two=2, c=65)
        nc.gpsimd.memset(vt4[:, :, :, 64:65], 1.0)

        # graduated input chunks split across both HWDGE queues
        qi = [0]

        def ldq(dst_ap, src_ap):
            eng = nc.sync if qi[0] % 2 == 0 else nc.scalar
            eng.dma_start(out=dst_ap, in_=src_ap)
            qi[0] += 1

        for li, (t0, t1) in enumerate(((0, 1), (1, 2), (2, 4), (4, 8))):
            for src_t, dst in ((x1T, x1sb), (x2T, x2sb), (xT, xsb)):
                src3 = src_t.rearrange("(g p) c -> p g c", g=2)[:, :, t0 * NT:t1 * NT]
                dst3 = dst.rearrange("p (g c) -> p g c", g=2)[:, :, t0 * NT:t1 * NT]
                ldq(dst3, src3)
            if li == 0:
                nc.sync.dma_start(out=wcat[:, 512:1024],
                                  in_=wall[4:8].rearrange("s p c -> p s c"))
                nc.scalar.dma_start(out=bcat[:], in_=ball.rearrange("s p -> p s"))

        small = ctx.enter_context(tc.tile_pool(name="small", bufs=2))
        pout = ctx.enter_context(tc.tile_pool(name="pout", bufs=4))
        poolO = ctx.enter_context(tc.tile_pool(name="poolO", bufs=2, space="PSUM"))
        poolEA = ctx.enter_context(tc.tile_pool(name="poolEA", bufs=6))
        poolED = ctx.enter_context(tc.tile_pool(name="poolED", bufs=6))

        def exp_slice(sp_ap, key, on_act, ets):
            if on_act:
                et = poolEA.tile([128, NT], BF16, tag="eta", name=f"eta_{key}")
                nc.scalar.activation(et[:], sp_ap, AF.Exp, scale=0.125)
                ets[key] = et[:]
            else:
                et = poolED.tile([128, NT], I16, tag="etd", name=f"etd_{key}")
                nc.vector.tensor_scalar(et[:], sp_ap, float(EA), float(EB),
                                        ALU.mult, ALU.add)
                ets[key] = et.bitcast(BF16)[:]

        def pv_slice(op, h, key, m, ets):
            vs = slice(m * 130 + 65 * h, m * 130 + 65 * h + 65)
            nc.tensor.matmul(op[:], Vtok[:, vs], ets.pop(key),
                             start=(m == 0), stop=(m == 31))

        def normalize(op, j, h):
            qs = slice(j * NT, (j + 1) * NT)
            hs = slice(64 * h, 64 * (h + 1))
            rec = small.tile([1, NT], F32, tag="rec", name=f"rec_{j}_{h}")
            nc.vector.reciprocal(rec[:], op[64:65, :])
            rbs = small.tile([64, NT], F32, tag="rbs", name=f"rbs_{j}_{h}")
            nc.gpsimd.partition_broadcast(rbs[:], rec[:])
            nc.vector.tensor_mul(Ocat[hs, qs], op[0:64, :], rbs[:])

        # ------- merged phase: projections + attention tile j=0 -------
        with ExitStack() as actx:
            poolA = actx.enter_context(tc.tile_pool(name="poolA", bufs=2, space="PSUM"))
            poolV = actx.enter_context(tc.tile_pool(name="poolV", bufs=1, space="PSUM"))
            spM = actx.enter_context(tc.tile_pool(name="spM", bufs=3, space="PSUM"))

            opM = [poolO.tile([65, NT], F32, tag="op", name=f"op_0_{h}")
                   for h in range(2)]
            qs0 = slice(0, NT)

            def a_mm(t):
                cs = slice(t * NT, (t + 1) * NT)
                kq1p = poolA.tile([128, NT], F32, tag="mmA", name=f"kq1p_{t}")
                nc.tensor.matmul(kq1p[:], w_kq1[0], x1sb[:, cs], start=True, stop=False)
                nc.tensor.matmul(kq1p[:], w_kq1[1], x1sb[:, N:][:, cs], start=False, stop=True)
                kq2p = poolA.tile([128, NT], F32, tag="mmA", name=f"kq2p_{t}")
                nc.tensor.matmul(kq2p[:], w_kq2[0], x2sb[:, cs], start=True, stop=False)
                nc.tensor.matmul(kq2p[:], w_kq2[1], x2sb[:, N:][:, cs], start=False, stop=True)
                return kq1p, kq2p

            def kt_copies(t, kq1p, kq2p):
                cs = slice(t * NT, (t + 1) * NT)
                nc.scalar.copy(KT[0:32, cs], kq1p[0:32, :])
                nc.vector.tensor_copy(KT[32:64, cs], kq2p[0:32, :])
                nc.scalar.copy(KT[64:96, cs], kq1p[64:96, :])
                nc.vector.tensor_copy(KT[96:128, cs], kq2p[64:96, :])

            def qt_v_work(t, kq1p, kq2p):
                cs = slice(t * NT, (t + 1) * NT)
                if t < 4:
                    nc.scalar.activation(QT[0:32, cs], kq1p[32:64, :], AF.Identity, bias=b_q[0:32])
                    nc.vector.tensor_scalar_add(QT[32:64, cs], kq2p[32:64, :], b_q[32:64])
                    nc.scalar.activation(QT[64:96, cs], kq1p[96:128, :], AF.Identity, bias=b_q[64:96])
                    nc.vector.tensor_scalar_add(QT[96:128, cs], kq2p[96:128, :], b_q[96:128])
                vtp = poolV.tile([128, NT], F32, tag="vtp", name=f"vtp_{t}")
                for s in range(4):
                    ts_ = slice((4 * t + s) * 128, (4 * t + s + 1) * 128)
                    ms = slice(s * 128, (s + 1) * 128)
                    nc.tensor.matmul(vtp[:, ms], xsb[:, ts_], w_vT[0], start=True, stop=False)
                    nc.tensor.matmul(vtp[:, ms], xsb[:, N:][:, ts_], w_vT[1], start=False, stop=True)
                vtp4 = vtp.rearrange("p (m c) -> p m c", c=128)
                vt3 = Vtok.rearrange("p (m c) -> p m c", c=130)
                dst = vt3[:, 4 * t:4 * (t + 1), :]
                if t % 2 == 0:
                    nc.scalar.copy(dst[:, :, 0:64], vtp4[:, :, 0:64])
                    nc.vector.tensor_copy(dst[:, :, 65:129], vtp4[:, :, 64:128])
                else:
                    nc.vector.tensor_copy(dst[:, :, 0:64], vtp4[:, :, 0:64])
                    nc.scalar.copy(dst[:, :, 65:129], vtp4[:, :, 64:128])

            def s_slice_m(h, m, sps):
                sp = spM.tile([128, NT], F32, tag="sp", name=f"spM_{h}_{m}")
                sps[(h, m)] = sp
                hs = slice(64 * h, 64 * (h + 1))
                nc.tensor.matmul(sp[:], KT[hs, m * 128:(m + 1) * 128],
                                 QT[hs, qs0], start=True, stop=True)

            kq = a_mm(0)
            kt_copies(0, *kq)
            qt_v_work(0, *kq)
            sps = {}
            ets = {}
            # software-pipelined S one m ahead: exp(m) starts while PE does
            # S(m+1); PV(m) lands after. Next tile's projections + KT copies
            # are issued two slice-iterations in, so copies don't block exps.
            s_slice_m(0, 0, sps)
            s_slice_m(1, 0, sps)
            for t in range(8):
                for s in range(4):
                    m = 4 * t + s
                    if s == 2 and t < 7:
                        kq = a_mm(t + 1)
                        kt_copies(t + 1, *kq)
                    exp_slice(sps.pop((0, m))[:], (0, m), (m % 2) == 1, ets)
                    if m < 31:
                        s_slice_m(0, m + 1, sps)
                    exp_slice(sps.pop((1, m))[:], (1, m), (m % 2) == 0, ets)
                    if m < 31:
                        s_slice_m(1, m + 1, sps)
                    pv_slice(opM[0], 0, (0, m), m, ets)
                    pv_slice(opM[1], 1, (1, m), m, ets)
                if t < 7:
                    qt_v_work(t + 1, *kq)

            normalize(opM[0], 0, 0)

        pending_norm_m = [(opM[1], 0, 1)]

        # ------- remaining attention tiles j=1..3 -------
        with ExitStack() as bctx:
            poolS = bctx.enter_context(tc.tile_pool(name="poolS", bufs=5, space="PSUM"))
            poolP = bctx.enter_context(tc.tile_pool(name="poolP", bufs=1, space="PSUM"))

            pending_c = [0]
            pending_norm = pending_norm_m

            def issue_c(j, g):
                qs = slice(j * NT, (j + 1) * NT)
                pp = poolP.tile([128, NT], F32, tag="pp", name=f"pp_{j}_{g}")
                nc.tensor.matmul(pp[:], w_out[g], Ocat[:, qs], start=True, stop=True)
                osb = pout.tile([128, NT], BF16, tag="osb", name=f"osb_{j}_{g}")
                nc.vector.scalar_tensor_tensor(
                    osb[:], pp[:], b_out[g], xsb[:, g * N:][:, qs],
                    op0=ALU.add, op1=ALU.add)
                nc.sync.dma_start(out=outT[128 * g:128 * (g + 1), qs], in_=osb[:])

            segs = [(j, h) for j in range(1, 4) for h in range(2)]
            sps = {}
            ets = {}

            def s_slice(si, m):
                j, h = segs[si]
                hs = slice(64 * h, 64 * (h + 1))
                qs = slice(j * NT, (j + 1) * NT)
                sp = poolS.tile([128, NT], F32, tag="sp", name=f"sp_{j}_{h}_{m}")
                sps[(si, m)] = sp
                nc.tensor.matmul(sp[:], KT[hs, m * 128:(m + 1) * 128],
                                 QT[hs, qs], start=True, stop=True)

            LOOK = 4
            for m in range(LOOK):
                s_slice(0, m)
            for si, (j, h) in enumerate(segs):
                op = poolO.tile([65, NT], F32, tag="op", name=f"op_{j}_{h}")
                eng = _ENG_C if (h == 0 and pending_c[0] is not None) else _ENG
                for m in range(32):
                    # lookahead continues into the next segment's slices
                    tgt = m + LOOK
                    if tgt < 32:
                        s_slice(si, tgt)
                    elif si + 1 < len(segs):
                        s_slice(si + 1, tgt - 32)
                    if m == 1 and pending_norm:
                        normalize(*pending_norm.pop())
                    if m in (6, 14) and pending_c[0] is not None:
                        issue_c(pending_c[0], 0 if m == 6 else 1)
                        if m == 14:
                            pending_c[0] = None
                    exp_slice(sps.pop((si, m))[:], (si, m), eng[m], ets)
                    pv_slice(op, h, (si, m), m, ets)
                if h == 0:
                    normalize(op, j, h)
                else:
                    pending_norm.append((op, j, h))
                    pending_c[0] = j
            normalize(*pending_norm.pop())
            issue_c(pending_c[0], 0)
            issue_c(pending_c[0], 1)

    nc.compile()
    return nc


_NC = None


def _get_nc():
    global _NC
    if _NC is None:
        _NC = build_bass()
    return _NC


def kernel(**inputs):
    out, _ = _run(inputs, trace=False)
    return out


def _run(inputs, trace=False):
    eps = 1e-5
    f32 = np.float32
    bf16 = mybir.dt.np(BF16)
    inp = {k: np.asarray(v, dtype=np.float32) for k, v in inputs.items()}

    s1 = inp['bn1_g'] / np.sqrt(inp['bn1_v'] + eps)
    t1 = inp['bn1_b'] - inp['bn1_m'] * s1
    s2 = inp['bn2_g'] / np.sqrt(inp['bn2_v'] + eps)
    t2 = inp['bn2_b'] - inp['bn2_m'] * s2
    W1 = inp['kq1_w'] * s1[None, :]
    b1 = inp['kq1_b'] + inp['kq1_w'] @ t1
    W2 = inp['kq2_w'] * s2[None, :]
    b2 = inp['kq2_b'] + inp['kq2_w'] @ t2
    sl = inp['bnl_g'] / np.sqrt(inp['bnl_v'] + eps)
    tl = inp['bnl_b'] - inp['bnl_m'] * sl
    ws = inp['w_scale'][0]
    Wout = (ws * sl)[:, None] * inp['out_w']
    bout_f = ws * (sl * inp['out_b'] + tl)
    # V bias commutes out of softmax-weighted sum: fold into out bias
    bout_f = bout_f + Wout @ inp['v_b']

    WoT = Wout.T
    wall = np.concatenate([
        W1.T.reshape(2, 128, 128),
        W2.T.reshape(2, 128, 128),
        inp['v_w'].T.reshape(2, 128, 128),
        WoT.reshape(128, 2, 128).transpose(1, 0, 2),
    ], axis=0).astype(bf16)
    bq = np.concatenate([b1[32:64], b2[32:64], b1[96:128], b2[96:128]])
    ball = np.stack([bq, bout_f[0:128], bout_f[128:256]], axis=0).astype(f32)

    shared = dict(wall=np.ascontiguousarray(wall),
                  ball=np.ascontiguousarray(ball))

    in_maps = []
    for b in range(4):
        x1Tb = inp['x1'][b].reshape(C, N).astype(bf16)
        x2Tb = inp['x2'][b].reshape(C, N).astype(bf16)
        xTb = inp['x'][b].reshape(C, N).astype(bf16)
        for qh in range(2):
            if qh == 0:
                m = dict(x1T=x1Tb, x2T=x2Tb, xT=xTb)
            else:
                m = dict(x1T=np.roll(x1Tb, -NQ, axis=1),
                         x2T=np.roll(x2Tb, -NQ, axis=1),
                         xT=np.roll(xTb, -NQ, axis=1))
            m.update(shared)
            in_maps.append(m)

    nc = _get_nc()
    res = run_bass_kernel_spmd(nc, in_maps, list(range(8)), trace=trace)

    out = np.empty((4, C, 64, 64), dtype=f32)
    for b in range(4):
        full = np.empty((C, N), dtype=f32)
        full[:, 0:NQ] = res.results[2 * b]["outT"].astype(f32)
        full[:, NQ:N] = res.results[2 * b + 1]["outT"].astype(f32)
        out[b] = full.reshape(C, 64, 64)
    return out, res
